# revision 1
# baseline (speedup 1.0000x reference)
"""Bergman matrix layer TRN2 kernel (per-core program, batch-sharded).

Per core: hidden [T,1024] -> out [T,1024].
  m = hidden @ W_mat + b_mat                 (TensorE fp32, W streamed)
  M = m/(||m||_F + 1e-5)*4   per (t,h)       (ACT Square-accum, scale in-place)
  lr/rl unnormalized matvec chains on DVE    (exact, T steps, both dirs fused)
  emission: v = u/||u||  -> DRAM scratch
  out = gelu(concat(v_lr, v_rl) @ W_out + b_out)
"""

from contextlib import ExitStack

import concourse.bass as bass
import concourse.tile as tile
from concourse import mybir
from concourse.masks import make_identity

AF = mybir.ActivationFunctionType
ALU = mybir.AluOpType
F32 = mybir.dt.float32

HID = 1024
NH = 16
NCOLS = 4096
MAT_EPS = 1e-5
SQRT_MD = 4.0
RESCALE = 8192.0
RESCALE_EVERY = 256


def bcast_dim(ap, n, axis):
    """Insert a step-0 dim of size n at position `axis` of an AP."""
    dims = [list(d) for d in ap.ap]
    dims.insert(axis, [0, n])
    return bass.AP(tensor=ap.tensor, offset=ap.offset, ap=dims)


def rev_last(ap):
    """Reverse the last dim of an AP."""
    dims = [list(d) for d in ap.ap]
    step, cnt = dims[-1]
    off = ap.offset + step * (cnt - 1)
    dims[-1] = [-step, cnt]
    return bass.AP(tensor=ap.tensor, offset=off, ap=dims)


def build_kernel(ctx: ExitStack, tc: tile.TileContext, T: int, gelu=True):
    nc = tc.nc
    NT = T // 128
    NCH = T // 32
    NGRP = T // 128

    hidden = nc.dram_tensor("hidden", [T, HID], F32, kind="ExternalInput").ap()
    w_mat = nc.dram_tensor("w_mat", [HID, NCOLS], F32, kind="ExternalInput").ap()
    b_mat = nc.dram_tensor("b_mat", [1, NCOLS], F32, kind="ExternalInput").ap()
    w_out = nc.dram_tensor("w_out", [512, HID], F32, kind="ExternalInput").ap()
    b_out = nc.dram_tensor("b_out", [1, HID], F32, kind="ExternalInput").ap()
    out = nc.dram_tensor("out", [T, HID], F32, kind="ExternalOutput").ap()
    m_dram = nc.dram_tensor("m_scratch", [T, NCOLS], F32, kind="Internal").ap()

    singles = ctx.enter_context(tc.tile_pool(name="singles", bufs=1))
    hraw_p = ctx.enter_context(tc.tile_pool(name="hraw", bufs=2))
    ht_p = ctx.enter_context(tc.tile_pool(name="ht", bufs=2))
    w_p = ctx.enter_context(tc.tile_pool(name="wstr", bufs=4))
    mnorm_p = ctx.enter_context(tc.tile_pool(name="mnorm", bufs=4))
    s_p = ctx.enter_context(tc.tile_pool(name="stile", bufs=4))
    mscan_p = ctx.enter_context(tc.tile_pool(name="mscan", bufs=3))
    u_p = ctx.enter_context(tc.tile_pool(name="ugrp", bufs=2))
    prod_p = ctx.enter_context(tc.tile_pool(name="prod", bufs=2))
    x_p = ctx.enter_context(tc.tile_pool(name="xtile", bufs=2))
    osb_p = ctx.enter_context(tc.tile_pool(name="osb", bufs=2))
    dram_p = ctx.enter_context(tc.tile_pool(name="udram", bufs=NGRP, space="DRAM"))
    ps_tr = ctx.enter_context(tc.tile_pool(name="ps_tr", bufs=2, space="PSUM"))
    ps_mm = ctx.enter_context(tc.tile_pool(name="ps_mm", bufs=3, space="PSUM"))
    ps_out = ctx.enter_context(tc.tile_pool(name="ps_out", bufs=2, space="PSUM"))

    ident = singles.tile([128, 128], F32)
    make_identity(nc, ident)
    ones_row = singles.tile([1, 128], F32)
    nc.vector.memset(ones_row, 1.0)
    bmat_sb = singles.tile([1, NCOLS], F32)
    nc.sync.dma_start(out=bmat_sb, in_=b_mat)
    bout_sb = singles.tile([1, HID], F32)
    nc.sync.dma_start(out=bout_sb, in_=b_out)
    wout_sb = singles.tile([128, 4, HID], F32)
    for kt in range(4):
        nc.sync.dma_start(out=wout_sb[:, kt, :],
                          in_=w_out[kt * 128:(kt + 1) * 128, :])
    u_init = singles.tile([32, 16], F32)
    nc.vector.memset(u_init, 0.0)
    nc.vector.memset(u_init[:, 0:1], 1.0)

    mnorm_tiles = {}

    def phase1_pair(g):
        tts = [g] if g == NT - 1 - g else [g, NT - 1 - g]
        hts = {}
        for tt in tts:
            hraw = hraw_p.tile([128, HID], F32, tag="hraw")
            nc.sync.dma_start(out=hraw, in_=hidden[tt * 128:(tt + 1) * 128, :])
            ht = ht_p.tile([128, 8, 128], F32, tag="ht")
            for kc in range(8):
                ps = ps_tr.tile([128, 128], F32, tag="tr")
                nc.tensor.transpose(ps, hraw[:, kc * 128:(kc + 1) * 128], ident)
                nc.scalar.activation(ht[:, kc, :], ps, AF.Copy)
            hts[tt] = ht
        sts = {}
        for tt in tts:
            mnorm_tiles[tt] = mnorm_p.tile([128, NCOLS], F32, tag="mnorm", name=f"mnorm{tt}")
            sts[tt] = s_p.tile([128, NH], F32, tag="stile", name=f"st{tt}")
        for cg in range(8):
            col0 = cg * 512
            pss = {tt: ps_mm.tile([128, 512], F32, tag="mm", name=f"psmm{tt}_{cg}") for tt in tts}
            for kc in range(8):
                wsl = w_p.tile([128, 512], F32, tag="wstr")
                nc.sync.dma_start(
                    out=wsl, in_=w_mat[kc * 128:(kc + 1) * 128, col0:col0 + 512])
                for tt in tts:
                    nc.tensor.matmul(pss[tt], hts[tt][:, kc, :], wsl,
                                     start=(kc == 0), stop=False)
            for tt in tts:
                nc.tensor.matmul(pss[tt], ones_row, bmat_sb[:, col0:col0 + 512],
                                 start=False, stop=True)
            for tt in tts:
                ps, mn, st = pss[tt], mnorm_tiles[tt], sts[tt]
                for hh in range(2):
                    h = cg * 2 + hh
                    nc.scalar.activation(mn[:, h * 256:(h + 1) * 256],
                                         ps[:, hh * 256:(hh + 1) * 256], AF.Copy)
                    scr = prod_p.tile([128, 256], F32, tag="sq_scr")
                    nc.scalar.activation(scr, ps[:, hh * 256:(hh + 1) * 256],
                                         AF.Square, accum_out=st[:, h:h + 1])
        for tt in tts:
            st, mn = sts[tt], mnorm_tiles[tt]
            nc.scalar.activation(st, st, AF.Sqrt)
            nc.vector.tensor_scalar_add(st, st, MAT_EPS)
            nc.vector.reciprocal(st, st)
            nc.vector.tensor_scalar_mul(st, st, SQRT_MD)
            for h in range(NH):
                nc.vector.tensor_scalar_mul(mn[:, h * 256:(h + 1) * 256],
                                            mn[:, h * 256:(h + 1) * 256],
                                            st[:, h:h + 1])
            nc.sync.dma_start(out=m_dram[tt * 128:(tt + 1) * 128, :], in_=mn)

    CH = 16  # scan steps per mscan chunk

    def mscan_chunk(c):
        """[32=(d,h), CH tau, 256]; d=0: t=CH*c+tau, d=1: t=T-1-CH*c-tau."""
        mt = mscan_p.tile([32, CH, 256], F32, tag="mscan")
        t0 = CH * c
        thi = T - 1 - t0
        md = m_dram.rearrange("t (h x) -> t h x", h=16)
        # lr half: dims iterate (h, tau, x); in t = t0 + tau
        vl = md[t0:t0 + CH, :, :].transpose([1, 0, 2])
        nc.sync.dma_start(out=mt[0:16, :, :], in_=vl)
        # rl half: t = thi - tau (descending) -> negative t stride
        vr = md[thi - CH + 1:thi + 1, :, :].transpose([1, 0, 2])
        dims = [list(d) for d in vr.ap]
        step, cnt = dims[1]
        vrr = bass.AP(tensor=vr.tensor, offset=vr.offset + step * (cnt - 1),
                      ap=[dims[0], [-step, cnt], dims[2]])
        nc.sync.dma_start(out=mt[16:32, :, :], in_=vrr)
        return mt

    u_sb = {}       # grp -> sbuf tile (transient ring)
    u_dram = {}     # grp -> dram tile

    def scan_group(grp):
        ug = u_p.tile([32, 128, 16], F32, tag="ugrp")
        u_sb[grp] = ug
        for cc in range(128 // CH):
            c = grp * (128 // CH) + cc
            mt = mscan_chunk(c)
            for j in range(CH):
                tau = CH * c + j
                if tau == 0:
                    uprev = u_init
                elif cc == 0 and j == 0:
                    uprev = u_sb[grp - 1][:, 127, :]
                else:
                    uprev = ug[:, cc * CH + j - 1, :]
                upb = bcast_dim(uprev, 16, 1)      # [32, 16(bc), 16]
                prod = prod_p.tile([32, 16, 16], F32, tag="prod")
                m_in = mt[:, j, :].rearrange("p (i x) -> p i x", i=16)
                if tau % RESCALE_EVERY == 0 and tau > 0:
                    nc.vector.scalar_tensor_tensor(
                        prod, m_in, RESCALE, upb, op0=ALU.mult, op1=ALU.mult)
                else:
                    nc.vector.tensor_tensor(prod, m_in, upb, op=ALU.mult)
                nc.vector.reduce_sum(ug[:, cc * CH + j, :], prod,
                                     axis=mybir.AxisListType.X)

    def emit_group(grp):
        ug = u_sb[grp]
        nrm = s_p.tile([32, 128], F32, tag="nrm")
        for sb in range(4):
            sl = slice(sb * 32, (sb + 1) * 32)
            sq = prod_p.tile([32, 32, 16], F32, tag="sq_em")
            nc.vector.tensor_tensor(sq, ug[:, sl, :], ug[:, sl, :], op=ALU.mult)
            nc.vector.reduce_sum(nrm[:, sl], sq, axis=mybir.AxisListType.X)
        nc.scalar.activation(nrm, nrm, AF.Sqrt)
        nc.vector.reciprocal(nrm, nrm)
        nc.vector.tensor_tensor(ug, ug, bcast_dim(nrm, 16, 2), op=ALU.mult)
        ud = dram_p.tile([32, 128, 16], F32, tag=f"ud{grp}")
        u_dram[grp] = ud
        nc.sync.dma_start(out=ud, in_=ug)

    def out_block(b):
        """out rows [128b, 128b+128): lr from grp b (tau=t), rl from grp
        NGRP-1-b with tau = T-1-t (reversed)."""
        glr = u_dram[b]
        grl = u_dram[NGRP - 1 - b]
        xk = x_p.tile([128, 4, 128], F32, tag="xtile")   # [(h4 d i), kt, t]
        xv = xk.rearrange("(h4 d i) k t -> h4 d i k t", h4=4, d=2)
        for kt in range(4):
            for hh in range(4):
                h = kt * 4 + hh
                vl = glr.rearrange("(d h) t i -> d h t i", d=2)[0, h, :, :]
                nc.sync.dma_start(out=xv[hh, 0, :, kt, :],
                                  in_=vl.transpose([1, 0]))
                vr = grl.rearrange("(d h) t i -> d h t i", d=2)[1, h, :, :]
                nc.sync.dma_start(out=xv[hh, 1, :, kt, :],
                                  in_=rev_last(vr.transpose([1, 0])))
        for oc in range(2):
            ps = ps_out.tile([128, 512], F32, tag="po")
            for kt in range(4):
                nc.tensor.matmul(ps, xk[:, kt, :],
                                 wout_sb[:, kt, oc * 512:(oc + 1) * 512],
                                 start=(kt == 0), stop=False)
            nc.tensor.matmul(ps, ones_row, bout_sb[:, oc * 512:(oc + 1) * 512],
                             start=False, stop=True)
            osb = osb_p.tile([128, 512], F32, tag="osb")
            nc.scalar.activation(osb, ps, AF.Gelu if gelu else AF.Identity)
            nc.sync.dma_start(
                out=out[b * 128:(b + 1) * 128, oc * 512:(oc + 1) * 512],
                in_=osb)

    for g in range((NT + 1) // 2):
        phase1_pair(g)
    done = set()
    for grp in range(NGRP):
        scan_group(grp)
        emit_group(grp)
        mirror = NGRP - 1 - grp
        if mirror in u_dram and mirror not in done:
            done.add(grp); done.add(mirror)
            out_block(min(grp, mirror))
            if mirror != grp:
                out_block(max(grp, mirror))


def build_nc(T=2048, gelu=True):
    import concourse.bacc as bacc
    nc = bacc.Bacc("TRN2", target_bir_lowering=False, debug=False)
    with tile.TileContext(nc) as tc:
        with ExitStack() as ctx:
            build_kernel(ctx, tc, T, gelu=gelu)
    nc.compile()
    return nc




# ----------------------------------------------------------------------------
# Self-contained entry point: full inputs in, full outputs out (8 cores).
# ----------------------------------------------------------------------------
import numpy as np

_NC_CACHE = {}


def _get_nc(T):
    if T not in _NC_CACHE:
        _NC_CACHE[T] = build_nc(T=T, gelu=True)
    return _NC_CACHE[T]


def kernel(hidden_states, W_mat, b_mat, W_out, b_out):
    from concourse.bass_utils import run_bass_kernel_spmd
    B, T, _ = hidden_states.shape
    nc = _get_nc(T)
    w_mat = np.ascontiguousarray(W_mat, dtype=np.float32)
    b_mat_ = np.ascontiguousarray(b_mat, dtype=np.float32).reshape(1, -1)
    w_out = np.ascontiguousarray(W_out, dtype=np.float32)
    b_out_ = np.ascontiguousarray(b_out, dtype=np.float32).reshape(1, -1)
    in_maps = [
        {
            "hidden": np.ascontiguousarray(hidden_states[b], dtype=np.float32),
            "w_mat": w_mat,
            "b_mat": b_mat_,
            "w_out": w_out,
            "b_out": b_out_,
        }
        for b in range(B)
    ]
    res = run_bass_kernel_spmd(nc, in_maps, list(range(B)))
    return np.stack([res.results[b]["out"] for b in range(B)], axis=0)


# revision 6
# speedup vs baseline: 1.0477x; 1.0477x over previous
"""Bergman matrix layer TRN2 kernel (per-core program, batch-sharded).

Per core: hidden [T,1024] -> out [T,1024].
  m = hidden @ W_mat + b_mat                 (TensorE fp32, W streamed)
  M = m/(||m||_F + 1e-5)*4   per (t,h)       (ACT Square-accum, scale in-place)
  lr/rl unnormalized matvec chains on DVE    (exact, T steps, both dirs fused)
  emission: v = u/||u||  -> DRAM scratch
  out = gelu(concat(v_lr, v_rl) @ W_out + b_out)
"""

from contextlib import ExitStack

import concourse.bass as bass
import concourse.tile as tile
from concourse import mybir
from concourse.masks import make_identity


def _register_cum_matvec():
    import numpy as np
    from concourse.dve_spec import Spec, Src0, Src1, C1, scan, AluOp, lower
    from concourse.dve_uop import DveOpSpec
    import concourse.dve_ops as dve_ops
    from concourse.dve_ops import DveOp
    for op in dve_ops.OPS:
        if op.name == "CUM_MATVEC_ANT":
            return op

    def _ref(in0, in1, s0, s1, imm2):
        p = in0.shape[0]
        a = np.asarray(in0, dtype=np.float32).reshape(p, -1)
        b = np.asarray(in1, dtype=np.float32).reshape(p, -1)
        if isinstance(s1, np.ndarray):
            s1 = s1.reshape(p, -1)
        return np.cumsum(a * b * s1, axis=1).astype(np.float32)

    spec = Spec(body=scan(AluOp.ADD, Src0 * Src1 * C1), reference=_ref)
    op = DveOp("CUM_MATVEC_ANT", spec, subdim=False, uops_sha={})
    dve_ops.OPS.append(op)
    dve_ops._SUB_OPCODE_FOR_NAME[op.name] = (
        dve_ops._CUSTOM_DVE_ROW_BASE + len(dve_ops.OPS) - 1)
    if hasattr(dve_ops, "CUSTOM_DVE_SPECS"):
        dve_ops.CUSTOM_DVE_SPECS[op.name] = op.spec
    assert max(dve_ops._SUB_OPCODE_FOR_NAME.values()) < 0x20
    for ver in ("v3", "v4"):
        uops = lower(spec, ver=ver)
        opc = dve_ops.get_dve_sub_opcode(op.name)
        op.uops_sha[ver] = DveOpSpec(
            name=op.name, opcode=opc, uops=uops, rd1_en=True).sha(ver)
    return op


CUM_MATVEC = _register_cum_matvec()

AF = mybir.ActivationFunctionType
ALU = mybir.AluOpType
F32 = mybir.dt.float32

HID = 1024
NH = 16
NCOLS = 4096
MAT_EPS = 1e-5
SQRT_MD = 4.0
RESCALE = 8192.0
RESCALE_EVERY = 256


def bcast_dim(ap, n, axis):
    """Insert a step-0 dim of size n at position `axis` of an AP."""
    dims = [list(d) for d in ap.ap]
    dims.insert(axis, [0, n])
    return bass.AP(tensor=ap.tensor, offset=ap.offset, ap=dims)


def rev_last(ap):
    """Reverse the last dim of an AP."""
    dims = [list(d) for d in ap.ap]
    step, cnt = dims[-1]
    off = ap.offset + step * (cnt - 1)
    dims[-1] = [-step, cnt]
    return bass.AP(tensor=ap.tensor, offset=off, ap=dims)


def build_kernel(ctx: ExitStack, tc: tile.TileContext, T: int, gelu=True):
    nc = tc.nc
    NT = T // 128
    NCH = T // 32
    NGRP = T // 128

    hidden = nc.dram_tensor("hidden", [T, HID], F32, kind="ExternalInput").ap()
    w_mat = nc.dram_tensor("w_mat", [HID, NCOLS], F32, kind="ExternalInput").ap()
    b_mat = nc.dram_tensor("b_mat", [1, NCOLS], F32, kind="ExternalInput").ap()
    w_out = nc.dram_tensor("w_out", [512, HID], F32, kind="ExternalInput").ap()
    b_out = nc.dram_tensor("b_out", [1, HID], F32, kind="ExternalInput").ap()
    out = nc.dram_tensor("out", [T, HID], F32, kind="ExternalOutput").ap()
    m_dram = nc.dram_tensor("m_scratch", [T, NCOLS], F32, kind="Internal").ap()

    singles = ctx.enter_context(tc.tile_pool(name="singles", bufs=1))
    hraw_p = ctx.enter_context(tc.tile_pool(name="hraw", bufs=2))
    ht_p = ctx.enter_context(tc.tile_pool(name="ht", bufs=2))
    w_p = ctx.enter_context(tc.tile_pool(name="wstr", bufs=4))
    mnorm_p = ctx.enter_context(tc.tile_pool(name="mnorm", bufs=4))
    s_p = ctx.enter_context(tc.tile_pool(name="stile", bufs=4))
    mscan_p = ctx.enter_context(tc.tile_pool(name="mscan", bufs=3))
    u_p = ctx.enter_context(tc.tile_pool(name="ugrp", bufs=2))
    prod_p = ctx.enter_context(tc.tile_pool(name="prod", bufs=2))
    x_p = ctx.enter_context(tc.tile_pool(name="xtile", bufs=2))
    osb_p = ctx.enter_context(tc.tile_pool(name="osb", bufs=2))
    d_p = ctx.enter_context(tc.tile_pool(name="dtile", bufs=2))
    scr_p = ctx.enter_context(tc.tile_pool(name="scr", bufs=2))
    u2_p = ctx.enter_context(tc.tile_pool(name="u2", bufs=2))
    dram_p = ctx.enter_context(tc.tile_pool(name="udram", bufs=NGRP, space="DRAM"))
    ps_tr = ctx.enter_context(tc.tile_pool(name="ps_tr", bufs=2, space="PSUM"))
    ps_mm = ctx.enter_context(tc.tile_pool(name="ps_mm", bufs=3, space="PSUM"))
    ps_out = ctx.enter_context(tc.tile_pool(name="ps_out", bufs=2, space="PSUM"))

    ident = singles.tile([128, 128], F32)
    make_identity(nc, ident)
    ones_row = singles.tile([1, 128], F32)
    nc.vector.memset(ones_row, 1.0)
    bmat_sb = singles.tile([1, NCOLS], F32)
    nc.sync.dma_start(out=bmat_sb, in_=b_mat)
    bout_sb = singles.tile([1, HID], F32)
    nc.sync.dma_start(out=bout_sb, in_=b_out)
    wout_sb = singles.tile([128, 4, HID], F32)
    for kt in range(4):
        nc.sync.dma_start(out=wout_sb[:, kt, :],
                          in_=w_out[kt * 128:(kt + 1) * 128, :])
    w_init = singles.tile([32, 16], F32)
    nc.vector.memset(w_init, 1.0)

    def phase1_pair(g):
        tts = [g] if g == NT - 1 - g else [g, NT - 1 - g]
        hts = {}
        for tt in tts:
            hraw = hraw_p.tile([128, HID], F32, tag="hraw")
            nc.sync.dma_start(out=hraw, in_=hidden[tt * 128:(tt + 1) * 128, :])
            ht = ht_p.tile([128, 8, 128], F32, tag="ht")
            for kc in range(8):
                ps = ps_tr.tile([128, 128], F32, tag="tr")
                nc.tensor.transpose(ps, hraw[:, kc * 128:(kc + 1) * 128], ident)
                nc.scalar.activation(ht[:, kc, :], ps, AF.Copy)
            hts[tt] = ht
        sts = {}
        for tt in tts:
            sts[tt] = s_p.tile([128, NH], F32, tag="stile", name=f"st{tt}")
        for cg in range(8):
            col0 = cg * 512
            pss = {tt: ps_mm.tile([128, 512], F32, tag="mm", name=f"psmm{tt}_{cg}") for tt in tts}
            for kc in range(8):
                wsl = w_p.tile([128, 512], F32, tag="wstr")
                nc.sync.dma_start(
                    out=wsl, in_=w_mat[kc * 128:(kc + 1) * 128, col0:col0 + 512])
                for tt in tts:
                    nc.tensor.matmul(pss[tt], hts[tt][:, kc, :], wsl,
                                     start=(kc == 0), stop=False)
            for tt in tts:
                nc.tensor.matmul(pss[tt], ones_row, bmat_sb[:, col0:col0 + 512],
                                 start=False, stop=True)
            for tt in tts:
                ps, st = pss[tt], sts[tt]
                mn = mnorm_p.tile([128, 512], F32, tag="mnorm",
                                  name=f"mn{tt}_{cg}")
                ssl = st[:, cg * 2:cg * 2 + 2]
                for hh in range(2):
                    nc.scalar.activation(mn[:, hh * 256:(hh + 1) * 256],
                                         ps[:, hh * 256:(hh + 1) * 256], AF.Copy)
                    scr = prod_p.tile([128, 256], F32, tag="sq_scr")
                    nc.scalar.activation(scr, ps[:, hh * 256:(hh + 1) * 256],
                                         AF.Square,
                                         accum_out=st[:, cg * 2 + hh:cg * 2 + hh + 1])
                nc.scalar.activation(ssl, ssl, AF.Sqrt)
                nc.vector.tensor_scalar_add(ssl, ssl, MAT_EPS)
                nc.vector.reciprocal(ssl, ssl)
                nc.vector.tensor_scalar_mul(ssl, ssl, SQRT_MD)
                for hh in range(2):
                    nc.vector.tensor_scalar_mul(
                        mn[:, hh * 256:(hh + 1) * 256],
                        mn[:, hh * 256:(hh + 1) * 256],
                        st[:, cg * 2 + hh:cg * 2 + hh + 1])
                dt_ = d_p.tile([128, 512], F32, tag="dtile", name=f"d{tt}_{cg}")
                nc.vector.tensor_tensor(dt_[:, 0:511], mn[:, 0:511],
                                        mn[:, 1:512], op=ALU.subtract)
                mn_j15 = mn.rearrange("p (a j) -> p a j", j=16)[:, :, 15]
                dt_j15 = dt_.rearrange("p (a j) -> p a j", j=16)[:, :, 15]
                nc.vector.tensor_copy(dt_j15, mn_j15)
                nc.sync.dma_start(
                    out=m_dram[tt * 128:(tt + 1) * 128, col0:col0 + 512],
                    in_=dt_)

    CH = 16  # scan steps per mscan chunk

    def mscan_chunk(c):
        """[32=(d,h), CH tau, 256]; d=0: t=CH*c+tau, d=1: t=T-1-CH*c-tau."""
        mt = mscan_p.tile([32, CH, 256], F32, tag="mscan")
        t0 = CH * c
        thi = T - 1 - t0
        md = m_dram.rearrange("t (h x) -> t h x", h=16)
        # lr half: dims iterate (h, tau, x); in t = t0 + tau
        vl = md[t0:t0 + CH, :, :].transpose([1, 0, 2])
        nc.sync.dma_start(out=mt[0:16, :, :], in_=vl)
        # rl half: t = thi - tau (descending) -> negative t stride
        vr = md[thi - CH + 1:thi + 1, :, :].transpose([1, 0, 2])
        dims = [list(d) for d in vr.ap]
        step, cnt = dims[1]
        vrr = bass.AP(tensor=vr.tensor, offset=vr.offset + step * (cnt - 1),
                      ap=[dims[0], [-step, cnt], dims[2]])
        nc.sync.dma_start(out=mt[16:32, :, :], in_=vrr)
        return mt

    u_sb = {}       # grp -> sbuf tile (transient ring)
    u_dram = {}     # grp -> dram tile

    def tail_view(scr_slice):
        """[32,256] prefix-stream slice -> [32, 16(bc), 16] segment-tail view."""
        dims = [list(d) for d in scr_slice.ap]
        base = [dims[0], [0, 16], [16 * dims[-1][0], 16]]
        return bass.AP(tensor=scr_slice.tensor,
                       offset=scr_slice.offset + 15 * dims[-1][0], ap=base)

    scan_state = {}

    def scan_group(grp):
        ug = u_p.tile([32, 128, 16], F32, tag="ugrp")
        u_sb[grp] = ug
        prev_scr = scan_state.get("prev_scr")
        for cc in range(128 // CH):
            c = grp * (128 // CH) + cc
            mt = mscan_chunk(c)
            scr = scr_p.tile([32, CH, 256], F32, tag="scr")
            for j in range(CH):
                tau = CH * c + j
                if tau == 0:
                    wb = bcast_dim(w_init, 16, 1)
                elif j == 0:
                    wb = tail_view(prev_scr[:, CH - 1, :])
                else:
                    wb = tail_view(scr[:, j - 1, :])
                m_in = mt[:, j, :].rearrange("p (i x) -> p i x", i=16)
                s1 = RESCALE if (tau % RESCALE_EVERY == 0 and tau > 0) else 1.0
                nc.vector._custom_dve(
                    CUM_MATVEC, out=scr[:, j, :].rearrange("p (i x) -> p i x", i=16),
                    in0=m_in, in1=wb, s1=s1)
            prev_scr = scr
            scan_state["prev_scr"] = scr
            # extract w states for emission: ug[:, cc*CH+t, i] = scr[:, t, i*16+15]
            wt = bass.AP(tensor=scr.tensor, offset=scr.offset + 15 * scr.ap[-1][0],
                         ap=[list(scr.ap[0]), list(scr.ap[1]),
                             [16 * scr.ap[-1][0], 16]])
            nc.vector.tensor_copy(ug[:, cc * CH:(cc + 1) * CH, :], wt)

    def emit_group(grp):
        wg = u_sb[grp]
        u2 = u2_p.tile([32, 128, 16], F32, tag="u2")
        nc.vector.tensor_tensor(u2[:, :, 1:16], wg[:, :, 1:16], wg[:, :, 0:15],
                                op=ALU.subtract)
        nc.vector.tensor_copy(u2[:, :, 0:1], wg[:, :, 0:1])
        nrm = s_p.tile([32, 128], F32, tag="nrm")
        for sb in range(4):
            sl = slice(sb * 32, (sb + 1) * 32)
            sq = prod_p.tile([32, 32, 16], F32, tag="sq_em")
            nc.vector.tensor_tensor(sq, u2[:, sl, :], u2[:, sl, :], op=ALU.mult)
            nc.vector.reduce_sum(nrm[:, sl], sq, axis=mybir.AxisListType.X)
        nc.scalar.activation(nrm, nrm, AF.Sqrt)
        nc.vector.reciprocal(nrm, nrm)
        nc.vector.tensor_tensor(u2, u2, bcast_dim(nrm, 16, 2), op=ALU.mult)
        ud = dram_p.tile([32, 128, 16], F32, tag=f"ud{grp}")
        u_dram[grp] = ud
        nc.sync.dma_start(out=ud, in_=u2)

    def out_block(b):
        """out rows [128b, 128b+128): lr from grp b (tau=t), rl from grp
        NGRP-1-b with tau = T-1-t (reversed)."""
        glr = u_dram[b]
        grl = u_dram[NGRP - 1 - b]
        xk = x_p.tile([128, 4, 128], F32, tag="xtile")   # [(h4 d i), kt, t]
        xv = xk.rearrange("(h4 d i) k t -> h4 d i k t", h4=4, d=2)
        for kt in range(4):
            for hh in range(4):
                h = kt * 4 + hh
                vl = glr.rearrange("(d h) t i -> d h t i", d=2)[0, h, :, :]
                nc.sync.dma_start(out=xv[hh, 0, :, kt, :],
                                  in_=vl.transpose([1, 0]))
                vr = grl.rearrange("(d h) t i -> d h t i", d=2)[1, h, :, :]
                nc.sync.dma_start(out=xv[hh, 1, :, kt, :],
                                  in_=rev_last(vr.transpose([1, 0])))
        for oc in range(2):
            ps = ps_out.tile([128, 512], F32, tag="po")
            for kt in range(4):
                nc.tensor.matmul(ps, xk[:, kt, :],
                                 wout_sb[:, kt, oc * 512:(oc + 1) * 512],
                                 start=(kt == 0), stop=False)
            nc.tensor.matmul(ps, ones_row, bout_sb[:, oc * 512:(oc + 1) * 512],
                             start=False, stop=True)
            osb = osb_p.tile([128, 512], F32, tag="osb")
            nc.scalar.activation(osb, ps, AF.Gelu if gelu else AF.Identity)
            nc.sync.dma_start(
                out=out[b * 128:(b + 1) * 128, oc * 512:(oc + 1) * 512],
                in_=osb)

    for g in range((NT + 1) // 2):
        phase1_pair(g)
    done = set()
    for grp in range(NGRP):
        scan_group(grp)
        emit_group(grp)
        mirror = NGRP - 1 - grp
        if mirror in u_dram and mirror not in done:
            done.add(grp); done.add(mirror)
            out_block(min(grp, mirror))
            if mirror != grp:
                out_block(max(grp, mirror))


def build_nc(T=2048, gelu=True):
    import concourse.bacc as bacc
    nc = bacc.Bacc("TRN2", target_bir_lowering=False, debug=False)
    with tile.TileContext(nc) as tc:
        with ExitStack() as ctx:
            build_kernel(ctx, tc, T, gelu=gelu)
    nc.compile()
    return nc




# ----------------------------------------------------------------------------
# Self-contained entry point: full inputs in, full outputs out (8 cores).
# ----------------------------------------------------------------------------
import numpy as np

_NC_CACHE = {}


def _get_nc(T):
    if T not in _NC_CACHE:
        _NC_CACHE[T] = build_nc(T=T, gelu=True)
    return _NC_CACHE[T]


def kernel(hidden_states, W_mat, b_mat, W_out, b_out):
    from concourse.bass_utils import run_bass_kernel_spmd
    B, T, _ = hidden_states.shape
    nc = _get_nc(T)
    w_mat = np.ascontiguousarray(W_mat, dtype=np.float32)
    b_mat_ = np.ascontiguousarray(b_mat, dtype=np.float32).reshape(1, -1)
    w_out = np.ascontiguousarray(W_out, dtype=np.float32)
    b_out_ = np.ascontiguousarray(b_out, dtype=np.float32).reshape(1, -1)
    in_maps = [
        {
            "hidden": np.ascontiguousarray(hidden_states[b], dtype=np.float32),
            "w_mat": w_mat,
            "b_mat": b_mat_,
            "w_out": w_out,
            "b_out": b_out_,
        }
        for b in range(B)
    ]
    res = run_bass_kernel_spmd(nc, in_maps, list(range(B)))
    return np.stack([res.results[b]["out"] for b in range(B)], axis=0)


# revision 8
# speedup vs baseline: 2193.8108x; 2093.9924x over previous
"""Bergman matrix layer TRN2 kernel (per-core program, batch-sharded).

Per core: hidden [T,1024] -> out [T,1024].
  m = hidden @ W_mat + b_mat                 (TensorE fp32, W streamed)
  M = m/(||m||_F + 1e-5)*4   per (t,h)       (ACT Square-accum, scale in-place)
  lr/rl unnormalized matvec chains on DVE    (exact, T steps, both dirs fused)
  emission: v = u/||u||  -> DRAM scratch
  out = gelu(concat(v_lr, v_rl) @ W_out + b_out)
"""

from contextlib import ExitStack

import concourse.bass as bass
import concourse.tile as tile
from concourse import mybir
from concourse.masks import make_identity


def _register_cum_matvec():
    import numpy as np
    from concourse.dve_spec import Spec, Src0, Src1, C1, scan, AluOp, lower
    from concourse.dve_uop import DveOpSpec
    import concourse.dve_ops as dve_ops
    from concourse.dve_ops import DveOp
    for op in dve_ops.OPS:
        if op.name == "CUM_MATVEC_ANT":
            return op

    def _ref(in0, in1, s0, s1, imm2):
        p = in0.shape[0]
        a = np.asarray(in0, dtype=np.float32).reshape(p, -1)
        b = np.asarray(in1, dtype=np.float32).reshape(p, -1)
        if isinstance(s1, np.ndarray):
            s1 = s1.reshape(p, -1)
        return np.cumsum(a * b * s1, axis=1).astype(np.float32)

    spec = Spec(body=scan(AluOp.ADD, Src0 * Src1 * C1), reference=_ref)
    op = DveOp("CUM_MATVEC_ANT", spec, subdim=False, uops_sha={})
    dve_ops.OPS.append(op)
    dve_ops._SUB_OPCODE_FOR_NAME[op.name] = (
        dve_ops._CUSTOM_DVE_ROW_BASE + len(dve_ops.OPS) - 1)
    if hasattr(dve_ops, "CUSTOM_DVE_SPECS"):
        dve_ops.CUSTOM_DVE_SPECS[op.name] = op.spec
    assert max(dve_ops._SUB_OPCODE_FOR_NAME.values()) < 0x20
    for ver in ("v3", "v4"):
        uops = lower(spec, ver=ver)
        opc = dve_ops.get_dve_sub_opcode(op.name)
        op.uops_sha[ver] = DveOpSpec(
            name=op.name, opcode=opc, uops=uops, rd1_en=True).sha(ver)
    return op


CUM_MATVEC = _register_cum_matvec()

AF = mybir.ActivationFunctionType
ALU = mybir.AluOpType
F32 = mybir.dt.float32
F32R = mybir.dt.float32r


def _r(ap):
    return ap.bitcast(F32R)

HID = 1024
NH = 16
NCOLS = 4096
MAT_EPS = 1e-5
SQRT_MD = 4.0
RESCALE = 8192.0
RESCALE_EVERY = 256


def bcast_dim(ap, n, axis):
    """Insert a step-0 dim of size n at position `axis` of an AP."""
    dims = [list(d) for d in ap.ap]
    dims.insert(axis, [0, n])
    return bass.AP(tensor=ap.tensor, offset=ap.offset, ap=dims)


def rev_last(ap):
    """Reverse the last dim of an AP."""
    dims = [list(d) for d in ap.ap]
    step, cnt = dims[-1]
    off = ap.offset + step * (cnt - 1)
    dims[-1] = [-step, cnt]
    return bass.AP(tensor=ap.tensor, offset=off, ap=dims)


def build_kernel(ctx: ExitStack, tc: tile.TileContext, T: int, gelu=True):
    nc = tc.nc
    NT = T // 128
    NCH = T // 32
    NGRP = T // 128

    hidden = nc.dram_tensor("hidden", [T, HID], F32, kind="ExternalInput").ap()
    w_mat = nc.dram_tensor("w_mat", [HID, NCOLS], F32, kind="ExternalInput").ap()
    b_mat = nc.dram_tensor("b_mat", [1, NCOLS], F32, kind="ExternalInput").ap()
    w_out = nc.dram_tensor("w_out", [512, HID], F32, kind="ExternalInput").ap()
    b_out = nc.dram_tensor("b_out", [1, HID], F32, kind="ExternalInput").ap()
    out = nc.dram_tensor("out", [T, HID], F32, kind="ExternalOutput").ap()
    m_dram = nc.dram_tensor("m_scratch", [T, NCOLS], F32, kind="Internal").ap()

    singles = ctx.enter_context(tc.tile_pool(name="singles", bufs=1))
    hraw_p = ctx.enter_context(tc.tile_pool(name="hraw", bufs=2))
    ht_p = ctx.enter_context(tc.tile_pool(name="ht", bufs=2))
    w_p = ctx.enter_context(tc.tile_pool(name="wstr", bufs=4))
    mnorm_p = ctx.enter_context(tc.tile_pool(name="mnorm", bufs=4))
    s_p = ctx.enter_context(tc.tile_pool(name="stile", bufs=4))
    mscan_p = ctx.enter_context(tc.tile_pool(name="mscan", bufs=3))
    u_p = ctx.enter_context(tc.tile_pool(name="ugrp", bufs=2))
    prod_p = ctx.enter_context(tc.tile_pool(name="prod", bufs=2))
    x_p = ctx.enter_context(tc.tile_pool(name="xtile", bufs=2))
    osb_p = ctx.enter_context(tc.tile_pool(name="osb", bufs=2))
    d_p = ctx.enter_context(tc.tile_pool(name="dtile", bufs=2))
    scr_p = ctx.enter_context(tc.tile_pool(name="scr", bufs=2))
    u2_p = ctx.enter_context(tc.tile_pool(name="u2", bufs=2))
    dram_p = ctx.enter_context(tc.tile_pool(name="udram", bufs=NGRP, space="DRAM"))
    ps_tr = ctx.enter_context(tc.tile_pool(name="ps_tr", bufs=2, space="PSUM"))
    ps_mm = ctx.enter_context(tc.tile_pool(name="ps_mm", bufs=3, space="PSUM"))
    ps_out = ctx.enter_context(tc.tile_pool(name="ps_out", bufs=2, space="PSUM"))

    ident = singles.tile([128, 128], F32)
    make_identity(nc, ident)
    ones_row = singles.tile([1, 128], F32)
    nc.vector.memset(ones_row, 1.0)
    bmat_sb = singles.tile([1, NCOLS], F32)
    nc.sync.dma_start(out=bmat_sb, in_=b_mat)
    bout_sb = singles.tile([1, HID], F32)
    nc.sync.dma_start(out=bout_sb, in_=b_out)
    wout_sb = singles.tile([128, 4, HID], F32)
    for kt in range(4):
        nc.sync.dma_start(out=wout_sb[:, kt, :],
                          in_=w_out[kt * 128:(kt + 1) * 128, :])
    w_init = singles.tile([32, 16], F32)
    nc.vector.memset(w_init, 1.0)

    def phase1_pair(g):
        tts = [g] if g == NT - 1 - g else [g, NT - 1 - g]
        hts = {}
        for tt in tts:
            hraw = hraw_p.tile([128, HID], F32, tag="hraw")
            nc.sync.dma_start(out=hraw, in_=hidden[tt * 128:(tt + 1) * 128, :])
            ht = ht_p.tile([128, 8, 128], F32, tag="ht")
            for kc in range(8):
                ps = ps_tr.tile([128, 128], F32, tag="tr")
                nc.tensor.transpose(ps, hraw[:, kc * 128:(kc + 1) * 128], ident)
                nc.scalar.activation(ht[:, kc, :], ps, AF.Copy)
            hts[tt] = ht
        sts = {}
        for tt in tts:
            sts[tt] = s_p.tile([128, NH], F32, tag="stile", name=f"st{tt}")
        for cg in range(8):
            col0 = cg * 512
            pss = {tt: ps_mm.tile([128, 512], F32, tag="mm", name=f"psmm{tt}_{cg}") for tt in tts}
            for kc in range(8):
                wsl = w_p.tile([128, 512], F32, tag="wstr")
                nc.sync.dma_start(
                    out=wsl, in_=w_mat[kc * 128:(kc + 1) * 128, col0:col0 + 512])
                for tt in tts:
                    nc.tensor.matmul(pss[tt], hts[tt][:, kc, :], wsl,
                                     start=(kc == 0), stop=False)
            for tt in tts:
                nc.tensor.matmul(pss[tt], ones_row, bmat_sb[:, col0:col0 + 512],
                                 start=False, stop=True)
            for tt in tts:
                ps, st = pss[tt], sts[tt]
                mn = mnorm_p.tile([128, 512], F32, tag="mnorm",
                                  name=f"mn{tt}_{cg}")
                ssl = st[:, cg * 2:cg * 2 + 2]
                for hh in range(2):
                    nc.scalar.activation(mn[:, hh * 256:(hh + 1) * 256],
                                         ps[:, hh * 256:(hh + 1) * 256], AF.Copy)
                    scr = prod_p.tile([128, 256], F32, tag="sq_scr")
                    nc.scalar.activation(scr, ps[:, hh * 256:(hh + 1) * 256],
                                         AF.Square,
                                         accum_out=st[:, cg * 2 + hh:cg * 2 + hh + 1])
                nc.scalar.activation(ssl, ssl, AF.Sqrt)
                nc.vector.tensor_scalar_add(ssl, ssl, MAT_EPS)
                nc.vector.reciprocal(ssl, ssl)
                nc.vector.tensor_scalar_mul(ssl, ssl, SQRT_MD)
                for hh in range(2):
                    nc.vector.tensor_scalar_mul(
                        mn[:, hh * 256:(hh + 1) * 256],
                        mn[:, hh * 256:(hh + 1) * 256],
                        st[:, cg * 2 + hh:cg * 2 + hh + 1])
                dt_ = d_p.tile([128, 512], F32, tag="dtile", name=f"d{tt}_{cg}")
                nc.vector.tensor_tensor(dt_[:, 0:511], mn[:, 0:511],
                                        mn[:, 1:512], op=ALU.subtract)
                mn_j15 = mn.rearrange("p (a j) -> p a j", j=16)[:, :, 15]
                dt_j15 = dt_.rearrange("p (a j) -> p a j", j=16)[:, :, 15]
                nc.vector.tensor_copy(dt_j15, mn_j15)
                nc.sync.dma_start(
                    out=m_dram[tt * 128:(tt + 1) * 128, col0:col0 + 512],
                    in_=dt_)

    CH = 16  # scan steps per mscan chunk

    def mscan_chunk(c):
        """[32=(d,h), CH tau, 256]; d=0: t=CH*c+tau, d=1: t=T-1-CH*c-tau."""
        mt = mscan_p.tile([32, CH, 256], F32, tag="mscan")
        t0 = CH * c
        thi = T - 1 - t0
        md = m_dram.rearrange("t (h x) -> t h x", h=16)
        # lr half: dims iterate (h, tau, x); in t = t0 + tau
        vl = md[t0:t0 + CH, :, :].transpose([1, 0, 2])
        nc.sync.dma_start(out=mt[0:16, :, :], in_=vl)
        # rl half: t = thi - tau (descending) -> negative t stride
        vr = md[thi - CH + 1:thi + 1, :, :].transpose([1, 0, 2])
        dims = [list(d) for d in vr.ap]
        step, cnt = dims[1]
        vrr = bass.AP(tensor=vr.tensor, offset=vr.offset + step * (cnt - 1),
                      ap=[dims[0], [-step, cnt], dims[2]])
        nc.sync.dma_start(out=mt[16:32, :, :], in_=vrr)
        return mt

    u_sb = {}       # grp -> sbuf tile (transient ring)
    u_dram = {}     # grp -> dram tile

    def tail_view(scr_slice):
        """[32,256] prefix-stream slice -> [32, 16(bc), 16] segment-tail view."""
        dims = [list(d) for d in scr_slice.ap]
        base = [dims[0], [0, 16], [16 * dims[-1][0], 16]]
        return bass.AP(tensor=scr_slice.tensor,
                       offset=scr_slice.offset + 15 * dims[-1][0], ap=base)

    scan_state = {}

    def scan_group(grp):
        ug = u_p.tile([32, 128, 16], F32, tag="ugrp")
        u_sb[grp] = ug
        prev_scr = scan_state.get("prev_scr")
        for cc in range(128 // CH):
            c = grp * (128 // CH) + cc
            mt = mscan_chunk(c)
            scr = scr_p.tile([32, CH, 256], F32, tag="scr")
            for j in range(CH):
                tau = CH * c + j
                if tau == 0:
                    wb = bcast_dim(w_init, 16, 1)
                elif j == 0:
                    wb = tail_view(prev_scr[:, CH - 1, :])
                else:
                    wb = tail_view(scr[:, j - 1, :])
                m_in = mt[:, j, :].rearrange("p (i x) -> p i x", i=16)
                s1 = RESCALE if (tau % RESCALE_EVERY == 0 and tau > 0) else 1.0
                nc.vector._custom_dve(
                    CUM_MATVEC, out=scr[:, j, :].rearrange("p (i x) -> p i x", i=16),
                    in0=m_in, in1=wb, s1=s1)
            prev_scr = scr
            scan_state["prev_scr"] = scr
            # extract w states for emission: ug[:, cc*CH+t, i] = scr[:, t, i*16+15]
            wt = bass.AP(tensor=scr.tensor, offset=scr.offset + 15 * scr.ap[-1][0],
                         ap=[list(scr.ap[0]), list(scr.ap[1]),
                             [16 * scr.ap[-1][0], 16]])
            nc.vector.tensor_copy(ug[:, cc * CH:(cc + 1) * CH, :], wt)

    def emit_group(grp):
        wg = u_sb[grp]
        u2 = u2_p.tile([32, 128, 16], F32, tag="u2")
        nc.vector.tensor_tensor(u2[:, :, 1:16], wg[:, :, 1:16], wg[:, :, 0:15],
                                op=ALU.subtract)
        nc.vector.tensor_copy(u2[:, :, 0:1], wg[:, :, 0:1])
        nrm = s_p.tile([32, 128], F32, tag="nrm")
        for sb in range(4):
            sl = slice(sb * 32, (sb + 1) * 32)
            sq = prod_p.tile([32, 32, 16], F32, tag="sq_em")
            nc.vector.tensor_tensor(sq, u2[:, sl, :], u2[:, sl, :], op=ALU.mult)
            nc.vector.reduce_sum(nrm[:, sl], sq, axis=mybir.AxisListType.X)
        nc.scalar.activation(nrm, nrm, AF.Sqrt)
        nc.vector.reciprocal(nrm, nrm)
        nc.vector.tensor_tensor(u2, u2, bcast_dim(nrm, 16, 2), op=ALU.mult)
        ud = dram_p.tile([32, 128, 16], F32, tag=f"ud{grp}")
        u_dram[grp] = ud
        nc.sync.dma_start(out=ud, in_=u2)

    def out_block(b):
        """out rows [128b, 128b+128): lr from grp b (tau=t), rl from grp
        NGRP-1-b with tau = T-1-t (reversed)."""
        glr = u_dram[b]
        grl = u_dram[NGRP - 1 - b]
        xk = x_p.tile([128, 4, 128], F32, tag="xtile")   # [(h4 d i), kt, t]
        xv = xk.rearrange("(h4 d i) k t -> h4 d i k t", h4=4, d=2)
        for kt in range(4):
            for hh in range(4):
                h = kt * 4 + hh
                vl = glr.rearrange("(d h) t i -> d h t i", d=2)[0, h, :, :]
                nc.sync.dma_start(out=xv[hh, 0, :, kt, :],
                                  in_=vl.transpose([1, 0]))
                vr = grl.rearrange("(d h) t i -> d h t i", d=2)[1, h, :, :]
                nc.sync.dma_start(out=xv[hh, 1, :, kt, :],
                                  in_=rev_last(vr.transpose([1, 0])))
        for oc in range(2):
            ps = ps_out.tile([128, 512], F32, tag="po")
            for kt in range(4):
                nc.tensor.matmul(ps, xk[:, kt, :],
                                 wout_sb[:, kt, oc * 512:(oc + 1) * 512],
                                 start=(kt == 0), stop=False)
            nc.tensor.matmul(ps, ones_row, bout_sb[:, oc * 512:(oc + 1) * 512],
                             start=False, stop=True)
            osb = osb_p.tile([128, 512], F32, tag="osb")
            nc.scalar.activation(osb, ps, AF.Gelu if gelu else AF.Identity)
            nc.sync.dma_start(
                out=out[b * 128:(b + 1) * 128, oc * 512:(oc + 1) * 512],
                in_=osb)

    for g in range((NT + 1) // 2):
        phase1_pair(g)
    done = set()
    for grp in range(NGRP):
        scan_group(grp)
        emit_group(grp)
        mirror = NGRP - 1 - grp
        if mirror in u_dram and mirror not in done:
            done.add(grp); done.add(mirror)
            out_block(min(grp, mirror))
            if mirror != grp:
                out_block(max(grp, mirror))


def build_nc(T=2048, gelu=True):
    import concourse.bacc as bacc
    nc = bacc.Bacc("TRN2", target_bir_lowering=False, debug=False)
    with tile.TileContext(nc) as tc:
        with ExitStack() as ctx:
            build_kernel(ctx, tc, T, gelu=gelu)
    nc.compile()
    return nc




# ----------------------------------------------------------------------------
# Self-contained entry point: full inputs in, full outputs out (8 cores).
# ----------------------------------------------------------------------------
import numpy as np

_NC_CACHE = {}


def _get_nc(T):
    if T not in _NC_CACHE:
        _NC_CACHE[T] = build_nc(T=T, gelu=True)
    return _NC_CACHE[T]


def kernel(hidden_states, W_mat, b_mat, W_out, b_out):
    from concourse.bass_utils import run_bass_kernel_spmd
    B, T, _ = hidden_states.shape
    nc = _get_nc(T)
    w_mat = np.ascontiguousarray(W_mat, dtype=np.float32)
    b_mat_ = np.ascontiguousarray(b_mat, dtype=np.float32).reshape(1, -1)
    w_out = np.ascontiguousarray(W_out, dtype=np.float32)
    b_out_ = np.ascontiguousarray(b_out, dtype=np.float32).reshape(1, -1)
    in_maps = [
        {
            "hidden": np.ascontiguousarray(hidden_states[b], dtype=np.float32),
            "w_mat": w_mat,
            "b_mat": b_mat_,
            "w_out": w_out,
            "b_out": b_out_,
        }
        for b in range(B)
    ]
    res = run_bass_kernel_spmd(nc, in_maps, list(range(B)))
    return np.stack([res.results[b]["out"] for b in range(B)], axis=0)


# revision 16
# speedup vs baseline: 2614.6060x; 1.1918x over previous
"""Bergman matrix layer TRN2 kernel (per-core program, batch-sharded).

Per core: hidden [T,1024] -> out [T,1024].
  m = hidden @ W_mat + b_mat                 (TensorE fp32, W streamed)
  M = m/(||m||_F + 1e-5)*4   per (t,h)       (ACT Square-accum, scale in-place)
  lr/rl unnormalized matvec chains on DVE    (exact, T steps, both dirs fused)
  emission: v = u/||u||  -> DRAM scratch
  out = gelu(concat(v_lr, v_rl) @ W_out + b_out)
"""

from contextlib import ExitStack

import concourse.bass as bass
import concourse.tile as tile
from concourse import mybir
from concourse.masks import make_identity


def _register_cum_matvec():
    import numpy as np
    from concourse.dve_spec import Spec, Src0, Src1, C1, scan, AluOp, lower
    from concourse.dve_uop import DveOpSpec
    import concourse.dve_ops as dve_ops
    from concourse.dve_ops import DveOp
    for op in dve_ops.OPS:
        if op.name == "CUM_MATVEC_ANT":
            return op

    def _ref(in0, in1, s0, s1, imm2):
        p = in0.shape[0]
        a = np.asarray(in0, dtype=np.float32).reshape(p, -1)
        b = np.asarray(in1, dtype=np.float32).reshape(p, -1)
        if isinstance(s1, np.ndarray):
            s1 = s1.reshape(p, -1)
        return np.cumsum(a * b * s1, axis=1).astype(np.float32)

    spec = Spec(body=scan(AluOp.ADD, Src0 * Src1 * C1), reference=_ref)
    op = DveOp("CUM_MATVEC_ANT", spec, subdim=False, uops_sha={})
    dve_ops.OPS.append(op)
    dve_ops._SUB_OPCODE_FOR_NAME[op.name] = (
        dve_ops._CUSTOM_DVE_ROW_BASE + len(dve_ops.OPS) - 1)
    if hasattr(dve_ops, "CUSTOM_DVE_SPECS"):
        dve_ops.CUSTOM_DVE_SPECS[op.name] = op.spec
    assert max(dve_ops._SUB_OPCODE_FOR_NAME.values()) < 0x20
    for ver in ("v3", "v4"):
        uops = lower(spec, ver=ver)
        opc = dve_ops.get_dve_sub_opcode(op.name)
        op.uops_sha[ver] = DveOpSpec(
            name=op.name, opcode=opc, uops=uops, rd1_en=True).sha(ver)
    return op


CUM_MATVEC = _register_cum_matvec()

AF = mybir.ActivationFunctionType
ALU = mybir.AluOpType
F32 = mybir.dt.float32
F32R = mybir.dt.float32r


def _r(ap):
    return ap.bitcast(F32R)

HID = 1024
NH = 16
NCOLS = 4096
MAT_EPS = 1e-5
SQRT_MD = 4.0
RESCALE = 8192.0
RESCALE_EVERY = 256


def bcast_dim(ap, n, axis):
    """Insert a step-0 dim of size n at position `axis` of an AP."""
    dims = [list(d) for d in ap.ap]
    dims.insert(axis, [0, n])
    return bass.AP(tensor=ap.tensor, offset=ap.offset, ap=dims)


def rev_last(ap):
    """Reverse the last dim of an AP."""
    dims = [list(d) for d in ap.ap]
    step, cnt = dims[-1]
    off = ap.offset + step * (cnt - 1)
    dims[-1] = [-step, cnt]
    return bass.AP(tensor=ap.tensor, offset=off, ap=dims)


def build_kernel(ctx: ExitStack, tc: tile.TileContext, T: int, gelu=True):
    nc = tc.nc
    NT = T // 128
    NCH = T // 32
    NGRP = T // 128

    hidden = nc.dram_tensor("hidden", [T, HID], F32, kind="ExternalInput").ap()
    w_mat = nc.dram_tensor("w_mat", [HID, NCOLS], F32, kind="ExternalInput").ap()
    b_mat = nc.dram_tensor("b_mat", [1, NCOLS], F32, kind="ExternalInput").ap()
    w_out = nc.dram_tensor("w_out", [512, HID], F32, kind="ExternalInput").ap()
    b_out = nc.dram_tensor("b_out", [1, HID], F32, kind="ExternalInput").ap()
    out = nc.dram_tensor("out", [T, HID], F32, kind="ExternalOutput").ap()
    m_dram = nc.dram_tensor("m_scratch", [T, NCOLS], F32, kind="Internal").ap()

    singles = ctx.enter_context(tc.tile_pool(name="singles", bufs=1))
    hraw_p = ctx.enter_context(tc.tile_pool(name="hraw", bufs=2))
    ht_p = ctx.enter_context(tc.tile_pool(name="ht", bufs=2))
    w_p = ctx.enter_context(tc.tile_pool(name="wstr", bufs=4))
    mnorm_p = ctx.enter_context(tc.tile_pool(name="mnorm", bufs=4))
    s_p = ctx.enter_context(tc.tile_pool(name="stile", bufs=4))
    mscan_p = ctx.enter_context(tc.tile_pool(name="mscan", bufs=3))
    u_p = ctx.enter_context(tc.tile_pool(name="ugrp", bufs=2))
    prod_p = ctx.enter_context(tc.tile_pool(name="prod", bufs=2))
    x_p = ctx.enter_context(tc.tile_pool(name="xtile", bufs=2))
    osb_p = ctx.enter_context(tc.tile_pool(name="osb", bufs=2))
    d_p = ctx.enter_context(tc.tile_pool(name="dtile", bufs=2))
    scr_p = ctx.enter_context(tc.tile_pool(name="scr", bufs=2))
    u2_p = ctx.enter_context(tc.tile_pool(name="u2", bufs=2))
    dram_p = ctx.enter_context(tc.tile_pool(name="udram", bufs=NGRP, space="DRAM"))
    ps_tr = ctx.enter_context(tc.tile_pool(name="ps_tr", bufs=2, space="PSUM"))
    ps_mm = ctx.enter_context(tc.tile_pool(name="ps_mm", bufs=3, space="PSUM"))
    ps_out = ctx.enter_context(tc.tile_pool(name="ps_out", bufs=2, space="PSUM"))

    ident = singles.tile([128, 128], F32)
    make_identity(nc, ident)
    ones_row = singles.tile([1, 128], F32)
    nc.vector.memset(ones_row, 1.0)
    bmat_sb = singles.tile([1, NCOLS], F32)
    nc.sync.dma_start(out=bmat_sb, in_=b_mat)
    bout_sb = singles.tile([1, HID], F32)
    nc.sync.dma_start(out=bout_sb, in_=b_out)
    wout_sb = singles.tile([128, 4, HID], F32)
    for kt in range(4):
        nc.sync.dma_start(out=wout_sb[:, kt, :],
                          in_=w_out[kt * 128:(kt + 1) * 128, :])
    w_init = singles.tile([32, 16], F32)
    nc.vector.memset(w_init, 1.0)

    def phase1_pair(g):
        tts = [g] if g == NT - 1 - g else [g, NT - 1 - g]
        hts = {}
        for tt in tts:
            hraw = hraw_p.tile([128, HID], F32, tag="hraw")
            nc.sync.dma_start(out=hraw, in_=hidden[tt * 128:(tt + 1) * 128, :])
            ht = ht_p.tile([128, 8, 128], F32, tag="ht")
            for kc in range(8):
                ps = ps_tr.tile([128, 128], F32, tag="tr")
                nc.tensor.transpose(ps, hraw[:, kc * 128:(kc + 1) * 128], ident)
                nc.scalar.activation(ht[:, kc, :], ps, AF.Copy)
            hts[tt] = ht
        sts = {}
        for tt in tts:
            sts[tt] = s_p.tile([128, NH], F32, tag="stile", name=f"st{tt}")
        for cg in range(8):
            col0 = cg * 512
            pss = {tt: ps_mm.tile([128, 512], F32, tag="mm", name=f"psmm{tt}_{cg}") for tt in tts}
            for kc in range(8):
                wsl = w_p.tile([128, 512], F32, tag="wstr")
                nc.scalar.dma_start(
                    out=wsl, in_=w_mat[kc * 128:(kc + 1) * 128, col0:col0 + 512])
                for tt in tts:
                    nc.tensor.matmul(pss[tt], hts[tt][:, kc, :], wsl,
                                     start=(kc == 0), stop=False)
            for tt in tts:
                nc.tensor.matmul(pss[tt], ones_row, bmat_sb[:, col0:col0 + 512],
                                 start=False, stop=True)
            for tt in tts:
                ps, st = pss[tt], sts[tt]
                mn = mnorm_p.tile([128, 512], F32, tag="mnorm",
                                  name=f"mn{tt}_{cg}")
                ssl = st[:, cg * 2:cg * 2 + 2]
                for hh in range(2):
                    scr = prod_p.tile([128, 256], F32, tag="sq_scr")
                    nc.scalar.activation(scr, ps[:, hh * 256:(hh + 1) * 256],
                                         AF.Square,
                                         accum_out=st[:, cg * 2 + hh:cg * 2 + hh + 1])
                nc.scalar.activation(ssl, ssl, AF.Sqrt)
                nc.vector.tensor_scalar_add(ssl, ssl, MAT_EPS)
                nc.vector.reciprocal(ssl, ssl)
                nc.vector.tensor_scalar_mul(ssl, ssl, SQRT_MD)
                for hh in range(2):
                    nc.scalar.activation(mn[:, hh * 256:(hh + 1) * 256],
                                         ps[:, hh * 256:(hh + 1) * 256], AF.Copy,
                                         scale=st[:, cg * 2 + hh:cg * 2 + hh + 1])
                dt_ = d_p.tile([128, 512], F32, tag="dtile", name=f"d{tt}_{cg}")
                nc.gpsimd.tensor_tensor(dt_[:, 0:511], mn[:, 0:511],
                                        mn[:, 1:512], op=ALU.subtract)
                mn_j15 = mn.rearrange("p (a j) -> p a j", j=16)[:, :, 15]
                dt_j15 = dt_.rearrange("p (a j) -> p a j", j=16)[:, :, 15]
                nc.gpsimd.tensor_copy(dt_j15, mn_j15)
                nc.scalar.dma_start(
                    out=m_dram[tt * 128:(tt + 1) * 128, col0:col0 + 512],
                    in_=dt_)

    CH = 16  # scan steps per mscan chunk

    def mscan_chunk(c):
        """[32=(d,h), CH tau, 256]; d=0: t=CH*c+tau, d=1: t=T-1-CH*c-tau."""
        mt = mscan_p.tile([32, CH, 256], F32, tag="mscan")
        t0 = CH * c
        thi = T - 1 - t0
        md = m_dram.rearrange("t (h x) -> t h x", h=16)
        # lr half: dims iterate (h, tau, x); in t = t0 + tau
        vl = md[t0:t0 + CH, :, :].transpose([1, 0, 2])
        nc.gpsimd.dma_start(out=mt[0:16, :, :], in_=vl)
        # rl half: t = thi - tau (descending) -> negative t stride
        vr = md[thi - CH + 1:thi + 1, :, :].transpose([1, 0, 2])
        dims = [list(d) for d in vr.ap]
        step, cnt = dims[1]
        vrr = bass.AP(tensor=vr.tensor, offset=vr.offset + step * (cnt - 1),
                      ap=[dims[0], [-step, cnt], dims[2]])
        nc.gpsimd.dma_start(out=mt[16:32, :, :], in_=vrr)
        return mt

    u_sb = {}       # grp -> sbuf tile (transient ring)
    u_dram = {}     # grp -> dram tile

    def tail_view(scr_slice):
        """[32,256] prefix-stream slice -> [32, 16(bc), 16] segment-tail view."""
        dims = [list(d) for d in scr_slice.ap]
        base = [dims[0], [0, 16], [16 * dims[-1][0], 16]]
        return bass.AP(tensor=scr_slice.tensor,
                       offset=scr_slice.offset + 15 * dims[-1][0], ap=base)

    scan_state = {}

    def scan_group(grp):
        ug = u_p.tile([32, 128, 16], F32, tag="ugrp")
        u_sb[grp] = ug
        prev_scr = scan_state.get("prev_scr")
        for cc in range(128 // CH):
            c = grp * (128 // CH) + cc
            mt = mscan_chunk(c)
            scr = scr_p.tile([32, CH, 256], F32, tag="scr")
            for j in range(CH):
                tau = CH * c + j
                if tau == 0:
                    wb = bcast_dim(w_init, 16, 1)
                elif j == 0:
                    wb = tail_view(prev_scr[:, CH - 1, :])
                else:
                    wb = tail_view(scr[:, j - 1, :])
                m_in = mt[:, j, :].rearrange("p (i x) -> p i x", i=16)
                s1 = RESCALE if (tau % RESCALE_EVERY == 0 and tau > 0) else 1.0
                nc.vector._custom_dve(
                    CUM_MATVEC, out=scr[:, j, :].rearrange("p (i x) -> p i x", i=16),
                    in0=m_in, in1=wb, s1=s1)
            prev_scr = scr
            scan_state["prev_scr"] = scr
            # extract w states for emission: ug[:, cc*CH+t, i] = scr[:, t, i*16+15]
            wt = bass.AP(tensor=scr.tensor, offset=scr.offset + 15 * scr.ap[-1][0],
                         ap=[list(scr.ap[0]), list(scr.ap[1]),
                             [16 * scr.ap[-1][0], 16]])
            nc.vector.tensor_copy(ug[:, cc * CH:(cc + 1) * CH, :], wt)

    def emit_group(grp):
        wg = u_sb[grp]
        u2 = u2_p.tile([32, 128, 16], F32, tag="u2")
        nc.gpsimd.tensor_tensor(u2[:, :, 1:16], wg[:, :, 1:16], wg[:, :, 0:15],
                                op=ALU.subtract)
        nc.gpsimd.tensor_copy(u2[:, :, 0:1], wg[:, :, 0:1])
        nrm = s_p.tile([32, 128], F32, tag="nrm")
        for sb in range(4):
            sl = slice(sb * 32, (sb + 1) * 32)
            sq = prod_p.tile([32, 32, 16], F32, tag="sq_em")
            nc.vector.tensor_tensor(sq, u2[:, sl, :], u2[:, sl, :], op=ALU.mult)
            nc.vector.reduce_sum(nrm[:, sl], sq, axis=mybir.AxisListType.X)
        nc.scalar.activation(nrm, nrm, AF.Sqrt)
        nc.vector.reciprocal(nrm, nrm)
        nc.gpsimd.tensor_tensor(u2, u2, bcast_dim(nrm, 16, 2), op=ALU.mult)
        ud = dram_p.tile([32, 128, 16], F32, tag=f"ud{grp}")
        u_dram[grp] = ud
        nc.sync.dma_start(out=ud, in_=u2)

    def out_block(b):
        """out rows [128b, 128b+128): lr from grp b (tau=t), rl from grp
        NGRP-1-b with tau = T-1-t (reversed)."""
        glr = u_dram[b]
        grl = u_dram[NGRP - 1 - b]
        xk = x_p.tile([128, 4, 128], F32, tag="xtile")   # [(h4 d i), kt, t]
        xv = xk.rearrange("(h4 d i) k t -> h4 d i k t", h4=4, d=2)
        for kt in range(4):
            for hh in range(4):
                h = kt * 4 + hh
                vl = glr.rearrange("(d h) t i -> d h t i", d=2)[0, h, :, :]
                nc.sync.dma_start(out=xv[hh, 0, :, kt, :],
                                  in_=vl.transpose([1, 0]))
                vr = grl.rearrange("(d h) t i -> d h t i", d=2)[1, h, :, :]
                nc.sync.dma_start(out=xv[hh, 1, :, kt, :],
                                  in_=rev_last(vr.transpose([1, 0])))
        for oc in range(2):
            ps = ps_out.tile([128, 512], F32, tag="po")
            for kt in range(4):
                nc.tensor.matmul(ps, xk[:, kt, :],
                                 wout_sb[:, kt, oc * 512:(oc + 1) * 512],
                                 start=(kt == 0), stop=False)
            nc.tensor.matmul(ps, ones_row, bout_sb[:, oc * 512:(oc + 1) * 512],
                             start=False, stop=True)
            osb = osb_p.tile([128, 512], F32, tag="osb")
            nc.scalar.activation(osb, ps, AF.Gelu if gelu else AF.Identity)
            nc.sync.dma_start(
                out=out[b * 128:(b + 1) * 128, oc * 512:(oc + 1) * 512],
                in_=osb)

    for g in range((NT + 1) // 2):
        phase1_pair(g)
    done = set()
    for grp in range(NGRP):
        scan_group(grp)
        emit_group(grp)
        mirror = NGRP - 1 - grp
        if mirror in u_dram and mirror not in done:
            done.add(grp); done.add(mirror)
            out_block(min(grp, mirror))
            if mirror != grp:
                out_block(max(grp, mirror))


def build_nc(T=2048, gelu=True):
    import concourse.bacc as bacc
    nc = bacc.Bacc("TRN2", target_bir_lowering=False, debug=False)
    with tile.TileContext(nc) as tc:
        with ExitStack() as ctx:
            build_kernel(ctx, tc, T, gelu=gelu)
    nc.compile()
    return nc




# ----------------------------------------------------------------------------
# Self-contained entry point: full inputs in, full outputs out (8 cores).
# ----------------------------------------------------------------------------
import numpy as np

_NC_CACHE = {}


def _get_nc(T):
    if T not in _NC_CACHE:
        _NC_CACHE[T] = build_nc(T=T, gelu=True)
    return _NC_CACHE[T]


def kernel(hidden_states, W_mat, b_mat, W_out, b_out):
    from concourse.bass_utils import run_bass_kernel_spmd
    B, T, _ = hidden_states.shape
    nc = _get_nc(T)
    w_mat = np.ascontiguousarray(W_mat, dtype=np.float32)
    b_mat_ = np.ascontiguousarray(b_mat, dtype=np.float32).reshape(1, -1)
    w_out = np.ascontiguousarray(W_out, dtype=np.float32)
    b_out_ = np.ascontiguousarray(b_out, dtype=np.float32).reshape(1, -1)
    in_maps = [
        {
            "hidden": np.ascontiguousarray(hidden_states[b], dtype=np.float32),
            "w_mat": w_mat,
            "b_mat": b_mat_,
            "w_out": w_out,
            "b_out": b_out_,
        }
        for b in range(B)
    ]
    res = run_bass_kernel_spmd(nc, in_maps, list(range(B)))
    return np.stack([res.results[b]["out"] for b in range(B)], axis=0)
